# revision 34
# baseline (speedup 1.0000x reference)
"""Self-contained Trainium2 Bass kernel for NemotronH MTP MoE layer.

Expert-parallel over 8 NeuronCores: core c owns experts [8c, 8c+8).  The
shared-expert MLP is split 2x4 (2 token halves x 4 slices of 512 of the
2048 intermediate dims): splitting tokens halves both the x load and the
partial-output store of the shared path at the cost of +2MB of shared
weights -- net DMA win, and only 4 partials sum per token.

The DeepSeekV3-style gate runs host-side (tiny); tokens are dispatched
host-side into per-expert 128-token slot blocks with the combine weight
folded in as sqrt(w) (exact: relu^2 is degree-2 homogeneous).

Kernel layout choices are driven by the DMA cost model:
 - every DMA moves >=512B contiguous per descriptor (full-rate);
   host pre-blocks all tensors as [128-partition, contiguous-free].
 - expert weights w1/w2 and dispatched tokens xs are f8e3m4 (host
   scales WS/XS_S, compensated exactly in the relu scale and the
   psum->sbuf copy scale); shared weights, x, and outputs stay f16.
   Measured end-to-end max-rel error ~1.7e-2 vs the 2e-2 budget.
 - the up-projection computes [I-partition, token] tiles directly
   (stationary = w1 chunk, moving = token block), so the down-projection
   needs no PE transposes.
 - loads are issued on the SP sequencer, stores alternate between the
   Pool (SWDGE) and Act (HWDGE) sequencers: a store waiting on compute
   never blocks load dispatch, and descriptor generation never
   serializes the store tail.
 - expert pipeline is software-pipelined (down-proj of expert e issues
   after up-proj of e+1) to keep the PE ramped; the last expert's up
   runs early and its w2 arrives in H-quarters at the end of the load
   queue, so only one quarter down-projection trails the final load.
"""

import sys

sys.path.insert(0, "/opt/trn_rl_repo")

import numpy as np

# ---- problem constants (hardcoded per contract) ----
B, S, H = 2, 512, 2048
E, G, TOPK_G, K = 64, 8, 4, 6
I = 512
SH_I = 2048
RSF = 2.5
T = B * S  # 1024 tokens
N_CORES = 8
EL = E // N_CORES  # 8 experts per core
P = 128
KH = H // P  # 16 K-tiles over hidden
NI = I // P  # 4 I-planes
NH = H // 512  # 4 H-chunks of 512
# shared-MLP 2x4 split
T2 = T // 2  # tokens per core for the shared path
SQ = SH_I // 4  # shared-intermediate dims per core (512 = 4 planes)
NSQ = SQ // P  # 4
TB2 = T2 // P  # 4 shared token blocks

XS_S = 2.0  # scale folded into f8e3m4 dispatched tokens
WS = 32.0  # host scale folded into f8e3m4 w1/w2

_PROG_CACHE = {}


def _gate_numpy(x, gate_w, gate_bias):
    """noaux_tc gate: sigmoid+bias, group top-2 sum, top-4 groups, top-6."""
    logits = x @ gate_w.T
    scores = 1.0 / (1.0 + np.exp(-logits))
    scores_b = scores + gate_bias
    sb_g = scores_b.reshape(T, G, E // G)
    top2 = np.sort(sb_g, axis=-1)[..., -2:].sum(-1, dtype=np.float32)
    grp_idx = np.argsort(-top2, axis=-1, kind="stable")[:, :TOPK_G]
    grp_mask = np.zeros((T, G), np.float32)
    np.put_along_axis(grp_mask, grp_idx, 1.0, axis=1)
    expert_mask = np.repeat(grp_mask, E // G, axis=-1) > 0
    masked = np.where(expert_mask, scores_b, -np.inf)
    top_idx = np.argsort(-masked, axis=1, kind="stable")[:, :K]
    topw = np.take_along_axis(scores, top_idx, axis=1)
    topw = topw / (topw.sum(-1, keepdims=True, dtype=np.float32) + 1e-20) * RSF
    return top_idx, topw.astype(np.float32)


def _build_program(nslot):
    """Build + compile the SPMD Bass program. nslot = 128-row token-slot
    blocks per expert (1 unless some expert holds >128 tokens)."""
    import concourse.bass as bass
    import concourse.tile as tile
    from concourse import bacc, mybir

    f32 = mybir.dt.float32
    f16 = mybir.dt.float16
    f8e3 = mybir.dt.float8e3
    Relu = mybir.ActivationFunctionType.Relu
    Copy = mybir.ActivationFunctionType.Copy

    NV = EL * nslot  # virtual experts (one 128-token slot block each)

    nc = bacc.Bacc("TRN2", target_bir_lowering=False, debug=False, num_devices=N_CORES)

    # blocked DRAM layouts; see _prepare for the host-side index formulas.
    xt = nc.dram_tensor("xt", [P, KH, T2], f16, kind="ExternalInput").ap()
    xs = nc.dram_tensor("xs", [NV, P, KH, P], f8e3, kind="ExternalInput").ap()
    w1 = nc.dram_tensor("w1", [EL, P, KH, NI, P], f8e3, kind="ExternalInput").ap()
    w2 = nc.dram_tensor("w2", [EL, P, NI, H], f8e3, kind="ExternalInput").ap()
    su = nc.dram_tensor("su", [P, KH, NSQ, P], f16, kind="ExternalInput").ap()
    sd = nc.dram_tensor("sd", [P, NSQ, H], f16, kind="ExternalInput").ap()
    out = nc.dram_tensor("out", [T2, H], f16, kind="ExternalOutput").ap()
    yall = nc.dram_tensor("yall", [NV * P, H], f16, kind="ExternalOutput").ap()

    store_rr = [0]

    with tile.TileContext(nc) as tc:
        with (
            tc.tile_pool(name="p_xt", bufs=1) as p_xt,
            tc.tile_pool(name="p_su", bufs=1) as p_su,
            tc.tile_pool(name="p_sd", bufs=1) as p_sd,
            tc.tile_pool(name="p_ash", bufs=1) as p_ash,
            tc.tile_pool(name="p_xs", bufs=6) as p_xs,
            tc.tile_pool(name="p_w1", bufs=5) as p_w1,
            tc.tile_pool(name="p_w2", bufs=4) as p_w2,
            tc.tile_pool(name="p_last", bufs=1) as p_last,
            tc.tile_pool(name="p_r", bufs=3) as p_r,
            tc.tile_pool(name="p_act", bufs=4) as p_act,
            tc.tile_pool(name="p_y", bufs=3) as p_y,
            tc.tile_pool(name="ps_main", bufs=5, space="PSUM") as ps_main,
            tc.tile_pool(name="ps_up", bufs=3, space="PSUM") as ps_up,
        ):
            # ---------------- loads for the shared MLP (SP queue) --------
            # su/xt in interleaved k-eighths: the shared up-projection
            # (the first PE work) starts as soon as the first sliver lands.
            xt_t = p_xt.tile([P, KH, T2], f16, name="xt")
            su_t = p_su.tile([P, KH, NSQ, P], f16, name="su")
            for kq in range(0, KH, 2):
                nc.sync.dma_start(su_t[:, kq : kq + 2], su[:, kq : kq + 2])
                nc.sync.dma_start(xt_t[:, kq : kq + 2], xt[:, kq : kq + 2])
            # the LAST expert's up-projection inputs load first; its w2
            # loads last (in quarters), so the only compute after the
            # final load is one quarter of a down-projection.
            EL_LAST = EL - 1
            nvl = EL_LAST * nslot
            xsL_ts = [
                p_last.tile([P, KH, P], f8e3, name="xsL%d" % s_)
                for s_ in range(nslot)
            ]
            w1L_t = p_last.tile([P, KH, NI, P], f8e3, name="w1L")
            w2L_t = p_last.tile([P, NI, H], f8e3, name="w2L")
            # sd is only needed by the shared-down blocks (interleaved from
            # expert 1 on); loading it after expert 0's prefetch lets the
            # expert stream run further ahead.
            sd_t = p_sd.tile([P, NSQ, H], f16, name="sd")

            # ---------------- shared up-projection -----------------------
            # psum [128 shI-sub, 512 tok] per m-plane; contraction over k.
            ps_sh = [ps_main.tile([P, T2], f32, name="psA") for _ in range(NSQ)]
            for k in range(KH):
                for m in range(NSQ):
                    nc.tensor.matmul(
                        ps_sh[m][:],
                        su_t[:, k, m, :],
                        xt_t[:, k, :],
                        start=(k == 0),
                        stop=(k == KH - 1),
                    )
            # relu^2 -> ash [128, NSQ(m), 512 tok] f16
            ash = p_ash.tile([P, NSQ, T2], f16, name="ash")
            for m in range(NSQ):
                pp = ps_sh[m]
                r = p_r.tile([P, T2], f32, name="r_sh")
                nc.scalar.activation(r[:], pp[:], Relu, 0.0, 1.0, 0.0)
                nc.vector.tensor_tensor(
                    out=ash[:, m, :], in0=pp[:], in1=r[:], op=mybir.AluOpType.mult
                )

            def store(dst, srcap):
                eng = nc.gpsimd if store_rr[0] % 2 == 0 else nc.scalar
                store_rr[0] += 1
                eng.dma_start(dst, srcap)

            # ---------------- routed experts (software-pipelined) --------
            def down_proj(act, w2_t, store_half, inv_s=1.0):
                """down-projection, hch-major so each 512-wide H chunk
                finishes (and can be copied+stored) as early as possible."""
                y = p_y.tile([P, H], f16, name="y_e")
                for hch in range(NH):
                    pd = ps_main.tile([P, 512], f32, name="psA")
                    for ip in range(NI):
                        nc.tensor.matmul(
                            pd[:],
                            act[:, ip, :],
                            w2_t[:, ip, hch * 512 : (hch + 1) * 512],
                            start=(ip == 0),
                            stop=(ip == NI - 1),
                        )
                    if hch % 2 == 0:
                        nc.vector.tensor_scalar_mul(
                            y[:, hch * 512 : (hch + 1) * 512], pd[:], inv_s
                        )
                    else:
                        nc.scalar.activation(
                            y[:, hch * 512 : (hch + 1) * 512], pd[:], Copy, 0.0,
                            inv_s,
                        )
                    if hch == 1:
                        store_half(y, 0)
                store_half(y, 1)

            def expert_down(st):
                v, act, w2_t = st
                down_proj(
                    act,
                    w2_t,
                    lambda y, hf: store(
                        yall[v * P : (v + 1) * P, hf * 1024 : (hf + 1) * 1024],
                        y[:, hf * 1024 : (hf + 1) * 1024],
                    ),
                    inv_s=1.0 / WS,
                )

            def shared_down_block(mt):
                """shared-down for token block mt; depends only on early
                loads, scheduled late to pack the DMA tail with its store."""
                down_proj(
                    ash[:, :, mt * P : (mt + 1) * P],
                    sd_t,
                    lambda o, hf: store(
                        out[mt * P : (mt + 1) * P, hf * 1024 : (hf + 1) * 1024],
                        o[:, hf * 1024 : (hf + 1) * 1024],
                    ),
                )

            def up_act(xs_t, w1_t, pool, name):
                """up-projection + relu^2 -> act [128 i-sub, NI, 128 tok]
                f16.  xs carries a XS_S scale, so pu = XS_S*up; the relu
                scale 1/XS_S^2 makes act = relu(up)^2 exactly."""
                pu = ps_up.tile([P, NI, P], f32, name="pu")
                for ci in range(NI):
                    for k in range(KH):
                        nc.tensor.matmul(
                            pu[:, ci, :],
                            w1_t[:, k, ci, :],
                            xs_t[:, k, :],
                            start=(k == 0),
                            stop=(k == KH - 1),
                        )
                r = p_r.tile([P, NI * P], f32, name="r_e")
                nc.scalar.activation(
                    r[:],
                    pu[:].rearrange("p a b -> p (a b)"),
                    Relu,
                    0.0,
                    1.0 / (WS * WS * XS_S * XS_S),
                    0.0,
                )
                act = pool.tile([P, NI, P], f16, name=name)
                nc.vector.tensor_tensor(
                    out=act[:].rearrange("p a b -> p (a b)"),
                    in0=pu[:].rearrange("p a b -> p (a b)"),
                    in1=r[:],
                    op=mybir.AluOpType.mult,
                )
                return act

            actL = []
            staged = None
            for e in range(EL_LAST):
                xs_ts = []
                for s_ in range(nslot):
                    xs_t = p_xs.tile([P, KH, P], f8e3, name="xs")
                    nc.sync.dma_start(xs_t[:], xs[e * nslot + s_])
                    xs_ts.append(xs_t)
                w1_t = p_w1.tile([P, KH, NI, P], f8e3, name="w1")
                nc.sync.dma_start(w1_t[:], w1[e])
                # w2 in two H-halves so the hch-major down-projection can
                # start after the first half lands.
                w2_t = p_w2.tile([P, NI, H], f8e3, name="w2")
                nc.sync.dma_start(w2_t[:, :, 0:1024], w2[e][:, :, 0:1024])
                nc.sync.dma_start(w2_t[:, :, 1024:2048], w2[e][:, :, 1024:2048])
                if e == 0:
                    # the LAST expert's up inputs ride along with expert
                    # 0's loads; its up-projection runs between up_0 and
                    # up_1, exactly covering the wait for w1_1.
                    for s_ in range(nslot):
                        nc.sync.dma_start(xsL_ts[s_][:], xs[nvl + s_])
                    nc.sync.dma_start(w1L_t[:], w1[EL_LAST])
                if e == 1:
                    nc.sync.dma_start(sd_t[:], sd)
                    for s_ in range(nslot):
                        actL.append(
                            up_act(xsL_ts[s_], w1L_t, p_last, "actL%d" % s_)
                        )
                if e == EL_LAST - 1:
                    # first half of the last expert's w2 rides along with
                    # the second-to-last expert's loads
                    for qh in range(2):
                        nc.sync.dma_start(
                            w2L_t[:, :, qh * 512 : (qh + 1) * 512],
                            w2[EL_LAST][:, :, qh * 512 : (qh + 1) * 512],
                        )
                for s_ in range(nslot):
                    v = e * nslot + s_
                    act = up_act(xs_ts[s_], w1_t, p_act, "act")
                    if staged is not None:
                        expert_down(staged)
                    staged = (v, act, w2_t)
                    # interleave the 4 shared-down blocks into the EARLY
                    # experts, where the PE is DMA-starved anyway: their
                    # stores queue up ready and pack the DMA tail, and no
                    # block compute sits after the final weight load.
                    blk = e - 2
                    if 0 <= blk < TB2 and s_ == nslot - 1:
                        shared_down_block(blk)
            if staged is not None:
                expert_down(staged)

            # last expert: the final two w2 H-quarters are the very last
            # loads; each quarter's down-projection chunk streams out as
            # soon as it lands.
            for qh in range(2, NH):
                nc.sync.dma_start(
                    w2L_t[:, :, qh * 512 : (qh + 1) * 512],
                    w2[EL_LAST][:, :, qh * 512 : (qh + 1) * 512],
                )
            for s_ in range(nslot):
                v = nvl + s_
                y = p_y.tile([P, H], f16, name="y_e")
                for hch in range(NH):
                    pd = ps_main.tile([P, 512], f32, name="psA")
                    for ip in range(NI):
                        nc.tensor.matmul(
                            pd[:],
                            actL[s_][:, ip, :],
                            w2L_t[:, ip, hch * 512 : (hch + 1) * 512],
                            start=(ip == 0),
                            stop=(ip == NI - 1),
                        )
                    if hch % 2 == 0:
                        nc.vector.tensor_scalar_mul(
                            y[:, hch * 512 : (hch + 1) * 512], pd[:], 1.0 / WS
                        )
                    else:
                        nc.scalar.activation(
                            y[:, hch * 512 : (hch + 1) * 512], pd[:], Copy, 0.0,
                            1.0 / WS,
                        )
                    store(
                        yall[v * P : (v + 1) * P, hch * 512 : (hch + 1) * 512],
                        y[:, hch * 512 : (hch + 1) * 512],
                    )

    nc.compile()
    return nc


def _prepare(inputs):
    """Host gate + dispatch: returns (nc, in_maps) ready for SPMD dispatch."""
    import ml_dtypes

    hidden_states = np.asarray(inputs["hidden_states"], dtype=np.float32)
    gate_w = np.asarray(inputs["gate_w"], dtype=np.float32)
    gate_bias = np.asarray(inputs["gate_bias"], dtype=np.float32)
    w1 = np.asarray(inputs["w1"], dtype=np.float32)
    w2 = np.asarray(inputs["w2"], dtype=np.float32)
    shared_up = np.asarray(inputs["shared_up"], dtype=np.float32)
    shared_down = np.asarray(inputs["shared_down"], dtype=np.float32)

    x = hidden_states.reshape(T, H)

    # ---- host gate + dispatch ----
    top_idx, topw = _gate_numpy(x, gate_w, gate_bias)
    sqw = np.sqrt(topw)

    tok_lists = [[] for _ in range(E)]
    scale_lists = [[] for _ in range(E)]
    for kk in range(K):
        for t in range(T):
            e = top_idx[t, kk]
            tok_lists[e].append(t)
            scale_lists[e].append(sqw[t, kk])
    counts = np.array([len(l) for l in tok_lists])
    nslot = max(1, int(np.ceil(counts.max() / P)))

    if nslot not in _PROG_CACHE:
        _PROG_CACHE[nslot] = _build_program(nslot)
    nc = _PROG_CACHE[nslot]

    NV = EL * nslot
    CAP = nslot * P

    in_maps = []
    for c in range(N_CORES):
        xs_b = np.zeros((NV, P, KH, P), ml_dtypes.float8_e3m4)
        for j in range(EL):
            e = c * EL + j
            toks = np.array(tok_lists[e], dtype=np.int64)
            scls = np.array(scale_lists[e], dtype=np.float32)
            n = len(toks)
            assert n <= CAP
            if n:
                xsp = np.zeros((CAP, H), np.float32)
                xsp[:n] = x[toks] * (scls[:, None] * XS_S)
                # [v within expert, p, k, c] = xsp[128v+c, 128k+p]
                xs_b[j * nslot : (j + 1) * nslot] = (
                    xsp.reshape(nslot, P, KH, P)
                    .transpose(0, 3, 2, 1)
                    .astype(ml_dtypes.float8_e3m4)
                )
        w1c = w1[c * EL : (c + 1) * EL]  # [EL, I, H]
        w2c = w2[c * EL : (c + 1) * EL]  # [EL, H, I]
        # w1 blocked: [e, p, k, ci, i] = w1[e, 128ci+i, 128k+p]
        w1_b = w1c.reshape(EL, NI, P, KH, P).transpose(0, 4, 3, 1, 2)
        # w2 blocked: [e, p, ip, h] = w2[e, h, 128ip+p]
        w2_b = w2c.reshape(EL, H, NI, P).transpose(0, 3, 2, 1)
        # shared 2x4 split: token half th, intermediate quarter q
        th, q = c // 4, c % 4
        cs = q * SQ
        # xt blocked: [p, k, t'] = x[512*th + t', 128k+p]
        xt_b = (
            x[th * T2 : (th + 1) * T2]
            .reshape(T2, KH, P)
            .transpose(2, 1, 0)
            .astype(np.float16)
        )
        # su blocked: [p, k, m, i] = shared_up[cs+128m+i, 128k+p]
        su_b = shared_up[cs : cs + SQ].reshape(NSQ, P, KH, P).transpose(3, 2, 0, 1)
        # sd blocked: [p, j, h] = shared_down[h, cs+128j+p]
        sd_b = shared_down[:, cs : cs + SQ].reshape(H, NSQ, P).transpose(2, 1, 0)
        in_maps.append(
            {
                "xt": np.ascontiguousarray(xt_b),
                "xs": np.ascontiguousarray(xs_b).view(np.uint8),
                "w1": np.ascontiguousarray(
                    (w1_b * WS).astype(ml_dtypes.float8_e3m4)
                ).view(np.uint8),
                "w2": np.ascontiguousarray(
                    (w2_b * WS).astype(ml_dtypes.float8_e3m4)
                ).view(np.uint8),
                "su": np.ascontiguousarray(su_b.astype(np.float16)),
                "sd": np.ascontiguousarray(sd_b.astype(np.float16)),
            }
        )

    return nc, in_maps, tok_lists, nslot


def _combine(results, tok_lists, nslot, out_shape, out_dtype):
    """Host unshard: sum shared partials + scatter-add routed expert rows."""
    CAP = nslot * P
    acc = np.zeros((T, H), np.float32)
    for c in range(N_CORES):
        th = c // 4
        acc[th * T2 : (th + 1) * T2] += results[c]["out"].astype(np.float32)
    for c in range(N_CORES):
        ya = results[c]["yall"].astype(np.float32)
        for j in range(EL):
            toks = tok_lists[c * EL + j]
            n = len(toks)
            if n:
                acc[toks] += ya[j * CAP : j * CAP + n]
    return acc.reshape(out_shape).astype(out_dtype)


def kernel(**inputs):
    from concourse.bass_utils import run_bass_kernel_spmd

    hidden_states = np.asarray(inputs["hidden_states"], dtype=np.float32)
    nc, in_maps, tok_lists, nslot = _prepare(inputs)
    res = run_bass_kernel_spmd(nc, in_maps, list(range(N_CORES)))
    return _combine(
        res.results, tok_lists, nslot, hidden_states.shape, hidden_states.dtype
    )


# revision 35
# speedup vs baseline: 1.0193x; 1.0193x over previous
"""Self-contained Trainium2 Bass kernel for NemotronH MTP MoE layer.

Expert-parallel over 8 NeuronCores: core c owns experts [8c, 8c+8).  The
shared-expert MLP is split 2x4 (2 token halves x 4 slices of 512 of the
2048 intermediate dims): splitting tokens halves both the x load and the
partial-output store of the shared path at the cost of +2MB of shared
weights -- net DMA win, and only 4 partials sum per token.

The DeepSeekV3-style gate runs host-side (tiny); tokens are dispatched
host-side into per-expert 128-token slot blocks with the combine weight
folded in as sqrt(w) (exact: relu^2 is degree-2 homogeneous).

Kernel layout choices are driven by the DMA cost model:
 - every DMA moves >=512B contiguous per descriptor (full-rate);
   host pre-blocks all tensors as [128-partition, contiguous-free].
 - expert weights w1/w2 and dispatched tokens xs are f8e3m4 (host
   scales WS/XS_S, compensated exactly in the relu scale and the
   psum->sbuf copy scale); shared weights, x, and outputs stay f16.
   Measured end-to-end max-rel error ~1.7e-2 vs the 2e-2 budget.
 - the up-projection computes [I-partition, token] tiles directly
   (stationary = w1 chunk, moving = token block), so the down-projection
   needs no PE transposes.
 - loads are issued on the SP sequencer, stores alternate between the
   Pool (SWDGE) and Act (HWDGE) sequencers: a store waiting on compute
   never blocks load dispatch, and descriptor generation never
   serializes the store tail.
 - expert pipeline is software-pipelined (down-proj of expert e issues
   after up-proj of e+1) to keep the PE ramped; the last expert's up
   runs early and its w2 arrives in H-quarters at the end of the load
   queue, so only one quarter down-projection trails the final load.
"""

import sys

sys.path.insert(0, "/opt/trn_rl_repo")

import numpy as np

# ---- problem constants (hardcoded per contract) ----
B, S, H = 2, 512, 2048
E, G, TOPK_G, K = 64, 8, 4, 6
I = 512
SH_I = 2048
RSF = 2.5
T = B * S  # 1024 tokens
N_CORES = 8
EL = E // N_CORES  # 8 experts per core
P = 128
KH = H // P  # 16 K-tiles over hidden
NI = I // P  # 4 I-planes
NH = H // 512  # 4 H-chunks of 512
# shared-MLP 2x4 split
T2 = T // 2  # tokens per core for the shared path
SQ = SH_I // 4  # shared-intermediate dims per core (512 = 4 planes)
NSQ = SQ // P  # 4
TB2 = T2 // P  # 4 shared token blocks

XS_S = 2.0  # scale folded into f8e3m4 dispatched tokens
WS = 32.0  # host scale folded into f8e3m4 w1/w2

_PROG_CACHE = {}


def _gate_numpy(x, gate_w, gate_bias):
    """noaux_tc gate: sigmoid+bias, group top-2 sum, top-4 groups, top-6."""
    logits = x @ gate_w.T
    scores = 1.0 / (1.0 + np.exp(-logits))
    scores_b = scores + gate_bias
    sb_g = scores_b.reshape(T, G, E // G)
    top2 = np.sort(sb_g, axis=-1)[..., -2:].sum(-1, dtype=np.float32)
    grp_idx = np.argsort(-top2, axis=-1, kind="stable")[:, :TOPK_G]
    grp_mask = np.zeros((T, G), np.float32)
    np.put_along_axis(grp_mask, grp_idx, 1.0, axis=1)
    expert_mask = np.repeat(grp_mask, E // G, axis=-1) > 0
    masked = np.where(expert_mask, scores_b, -np.inf)
    top_idx = np.argsort(-masked, axis=1, kind="stable")[:, :K]
    topw = np.take_along_axis(scores, top_idx, axis=1)
    topw = topw / (topw.sum(-1, keepdims=True, dtype=np.float32) + 1e-20) * RSF
    return top_idx, topw.astype(np.float32)


def _build_program(nslot):
    """Build + compile the SPMD Bass program. nslot = 128-row token-slot
    blocks per expert (1 unless some expert holds >128 tokens)."""
    import concourse.bass as bass
    import concourse.tile as tile
    from concourse import bacc, mybir

    f32 = mybir.dt.float32
    f16 = mybir.dt.float16
    f8e3 = mybir.dt.float8e3
    Relu = mybir.ActivationFunctionType.Relu
    Copy = mybir.ActivationFunctionType.Copy

    NV = EL * nslot  # virtual experts (one 128-token slot block each)

    nc = bacc.Bacc("TRN2", target_bir_lowering=False, debug=False, num_devices=N_CORES)

    # blocked DRAM layouts; see _prepare for the host-side index formulas.
    xt = nc.dram_tensor("xt", [P, KH, T2], f16, kind="ExternalInput").ap()
    xs = nc.dram_tensor("xs", [NV, P, KH, P], f8e3, kind="ExternalInput").ap()
    w1 = nc.dram_tensor("w1", [EL, P, KH, NI, P], f8e3, kind="ExternalInput").ap()
    w2 = nc.dram_tensor("w2", [EL, P, NI, H], f8e3, kind="ExternalInput").ap()
    su = nc.dram_tensor("su", [P, KH, NSQ, P], f16, kind="ExternalInput").ap()
    sd = nc.dram_tensor("sd", [P, NSQ, H], f16, kind="ExternalInput").ap()
    out = nc.dram_tensor("out", [T2, H], f16, kind="ExternalOutput").ap()
    yall = nc.dram_tensor("yall", [NV * P, H], f16, kind="ExternalOutput").ap()

    store_rr = [0]

    with tile.TileContext(nc) as tc:
        with (
            tc.tile_pool(name="p_xt", bufs=1) as p_xt,
            tc.tile_pool(name="p_su", bufs=1) as p_su,
            tc.tile_pool(name="p_sd", bufs=1) as p_sd,
            tc.tile_pool(name="p_ash", bufs=1) as p_ash,
            tc.tile_pool(name="p_xs", bufs=6) as p_xs,
            tc.tile_pool(name="p_w1", bufs=5) as p_w1,
            tc.tile_pool(name="p_w2", bufs=4) as p_w2,
            tc.tile_pool(name="p_last", bufs=1) as p_last,
            tc.tile_pool(name="p_r", bufs=3) as p_r,
            tc.tile_pool(name="p_act", bufs=4) as p_act,
            tc.tile_pool(name="p_y", bufs=3) as p_y,
            tc.tile_pool(name="ps_main", bufs=5, space="PSUM") as ps_main,
            tc.tile_pool(name="ps_up", bufs=3, space="PSUM") as ps_up,
        ):
            # ---------------- loads for the shared MLP (SP queue) --------
            # su/xt in interleaved k-eighths: the shared up-projection
            # (the first PE work) starts as soon as the first sliver lands.
            xt_t = p_xt.tile([P, KH, T2], f16, name="xt")
            su_t = p_su.tile([P, KH, NSQ, P], f16, name="su")
            for kq in range(0, KH, 2):
                nc.sync.dma_start(su_t[:, kq : kq + 2], su[:, kq : kq + 2])
                nc.sync.dma_start(xt_t[:, kq : kq + 2], xt[:, kq : kq + 2])
            # the LAST expert's up-projection inputs load first; its w2
            # loads last (in quarters), so the only compute after the
            # final load is one quarter of a down-projection.
            EL_LAST = EL - 1
            nvl = EL_LAST * nslot
            xsL_ts = [
                p_last.tile([P, KH, P], f8e3, name="xsL%d" % s_)
                for s_ in range(nslot)
            ]
            w1L_t = p_last.tile([P, KH, NI, P], f8e3, name="w1L")
            w2L_t = p_last.tile([P, NI, H], f8e3, name="w2L")
            # sd is only needed by the shared-down blocks (interleaved from
            # expert 1 on); loading it after expert 0's prefetch lets the
            # expert stream run further ahead.
            sd_t = p_sd.tile([P, NSQ, H], f16, name="sd")

            # ---------------- shared up-projection -----------------------
            # psum [128 shI-sub, 512 tok] per m-plane; contraction over k.
            ps_sh = [ps_main.tile([P, T2], f32, name="psA") for _ in range(NSQ)]
            for k in range(KH):
                for m in range(NSQ):
                    nc.tensor.matmul(
                        ps_sh[m][:],
                        su_t[:, k, m, :],
                        xt_t[:, k, :],
                        start=(k == 0),
                        stop=(k == KH - 1),
                    )
            # relu^2 -> ash [128, NSQ(m), 512 tok] f16
            ash = p_ash.tile([P, NSQ, T2], f16, name="ash")
            for m in range(NSQ):
                pp = ps_sh[m]
                r = p_r.tile([P, T2], f32, name="r_sh")
                nc.scalar.activation(r[:], pp[:], Relu, 0.0, 1.0, 0.0)
                nc.vector.tensor_tensor(
                    out=ash[:, m, :], in0=pp[:], in1=r[:], op=mybir.AluOpType.mult
                )

            def store(dst, srcap):
                eng = nc.gpsimd if store_rr[0] % 2 == 0 else nc.scalar
                store_rr[0] += 1
                eng.dma_start(dst, srcap)

            # ---------------- routed experts (software-pipelined) --------
            def down_proj(act, w2_t, store_half, inv_s=1.0):
                """down-projection, hch-major so each 512-wide H chunk
                finishes (and can be copied+stored) as early as possible."""
                y = p_y.tile([P, H], f16, name="y_e")
                for hch in range(NH):
                    pd = ps_main.tile([P, 512], f32, name="psA")
                    for ip in range(NI):
                        nc.tensor.matmul(
                            pd[:],
                            act[:, ip, :],
                            w2_t[:, ip, hch * 512 : (hch + 1) * 512],
                            start=(ip == 0),
                            stop=(ip == NI - 1),
                        )
                    if hch % 2 == 0:
                        nc.vector.tensor_scalar_mul(
                            y[:, hch * 512 : (hch + 1) * 512], pd[:], inv_s
                        )
                    else:
                        nc.scalar.activation(
                            y[:, hch * 512 : (hch + 1) * 512], pd[:], Copy, 0.0,
                            inv_s,
                        )
                    if hch == 1:
                        store_half(y, 0)
                store_half(y, 1)

            def expert_down(st):
                v, act, w2_t = st
                down_proj(
                    act,
                    w2_t,
                    lambda y, hf: store(
                        yall[v * P : (v + 1) * P, hf * 1024 : (hf + 1) * 1024],
                        y[:, hf * 1024 : (hf + 1) * 1024],
                    ),
                    inv_s=1.0 / WS,
                )

            def shared_down_block(mt):
                """shared-down for token block mt; depends only on early
                loads, scheduled late to pack the DMA tail with its store."""
                down_proj(
                    ash[:, :, mt * P : (mt + 1) * P],
                    sd_t,
                    lambda o, hf: store(
                        out[mt * P : (mt + 1) * P, hf * 1024 : (hf + 1) * 1024],
                        o[:, hf * 1024 : (hf + 1) * 1024],
                    ),
                )

            def up_act(xs_t, w1_t, pool, name):
                """up-projection + relu^2 -> act [128 i-sub, NI, 128 tok]
                f16.  xs carries a XS_S scale, so pu = XS_S*up; the relu
                scale 1/XS_S^2 makes act = relu(up)^2 exactly."""
                pu = ps_up.tile([P, NI, P], f32, name="pu")
                for ci in range(NI):
                    for k in range(KH):
                        nc.tensor.matmul(
                            pu[:, ci, :],
                            w1_t[:, k, ci, :],
                            xs_t[:, k, :],
                            start=(k == 0),
                            stop=(k == KH - 1),
                        )
                r = p_r.tile([P, NI * P], f32, name="r_e")
                nc.scalar.activation(
                    r[:],
                    pu[:].rearrange("p a b -> p (a b)"),
                    Relu,
                    0.0,
                    1.0 / (WS * WS * XS_S * XS_S),
                    0.0,
                )
                act = pool.tile([P, NI, P], f16, name=name)
                nc.vector.tensor_tensor(
                    out=act[:].rearrange("p a b -> p (a b)"),
                    in0=pu[:].rearrange("p a b -> p (a b)"),
                    in1=r[:],
                    op=mybir.AluOpType.mult,
                )
                return act

            actL = []
            staged = None
            w2_pending = [None]
            for e in range(EL_LAST):
                xs_ts = []
                for s_ in range(nslot):
                    xs_t = p_xs.tile([P, KH, P], f8e3, name="xs")
                    nc.sync.dma_start(xs_t[:], xs[e * nslot + s_])
                    xs_ts.append(xs_t)
                w1_t = p_w1.tile([P, KH, NI, P], f8e3, name="w1")
                nc.sync.dma_start(w1_t[:], w1[e])
                # w2_e loads one expert late (with e+1's other loads),
                # matching the down-projection stagger -- every byte lands
                # just before its first consumer.
                w2_t = p_w2.tile([P, NI, H], f8e3, name="w2")
                if e > 0:
                    pw2_t, pe_ = w2_pending[0]
                    nc.sync.dma_start(pw2_t[:, :, 0:1024], w2[pe_][:, :, 0:1024])
                    nc.sync.dma_start(
                        pw2_t[:, :, 1024:2048], w2[pe_][:, :, 1024:2048]
                    )
                w2_pending[0] = (w2_t, e)
                if e == 0:
                    # the LAST expert's up inputs ride along with expert
                    # 0's loads; its up-projection runs between up_0 and
                    # up_1, exactly covering the wait for w1_1.
                    for s_ in range(nslot):
                        nc.sync.dma_start(xsL_ts[s_][:], xs[nvl + s_])
                    nc.sync.dma_start(w1L_t[:], w1[EL_LAST])
                if e == 1:
                    nc.sync.dma_start(sd_t[:], sd)
                    for s_ in range(nslot):
                        actL.append(
                            up_act(xsL_ts[s_], w1L_t, p_last, "actL%d" % s_)
                        )
                if e == EL_LAST - 1:
                    # first half of the last expert's w2 rides along with
                    # the second-to-last expert's loads
                    for qh in range(2):
                        nc.sync.dma_start(
                            w2L_t[:, :, qh * 512 : (qh + 1) * 512],
                            w2[EL_LAST][:, :, qh * 512 : (qh + 1) * 512],
                        )
                for s_ in range(nslot):
                    v = e * nslot + s_
                    act = up_act(xs_ts[s_], w1_t, p_act, "act")
                    if staged is not None:
                        expert_down(staged)
                    staged = (v, act, w2_t)
                    if e == EL_LAST - 1 and s_ == nslot - 1:
                        # flush the final pending w2 load so the last
                        # staged expert's down-projection has its weights
                        pw2_t, pe_ = w2_pending[0]
                        nc.sync.dma_start(
                            pw2_t[:, :, 0:1024], w2[pe_][:, :, 0:1024]
                        )
                        nc.sync.dma_start(
                            pw2_t[:, :, 1024:2048], w2[pe_][:, :, 1024:2048]
                        )
                    # interleave the 4 shared-down blocks into the EARLY
                    # experts, where the PE is DMA-starved anyway: their
                    # stores queue up ready and pack the DMA tail, and no
                    # block compute sits after the final weight load.
                    blk = e - 2
                    if 0 <= blk < TB2 and s_ == nslot - 1:
                        shared_down_block(blk)
            if staged is not None:
                expert_down(staged)

            # last expert: the final two w2 H-quarters are the very last
            # loads; each quarter's down-projection chunk streams out as
            # soon as it lands.
            for qh in range(2, NH):
                nc.sync.dma_start(
                    w2L_t[:, :, qh * 512 : (qh + 1) * 512],
                    w2[EL_LAST][:, :, qh * 512 : (qh + 1) * 512],
                )
            for s_ in range(nslot):
                v = nvl + s_
                y = p_y.tile([P, H], f16, name="y_e")
                for hch in range(NH):
                    pd = ps_main.tile([P, 512], f32, name="psA")
                    for ip in range(NI):
                        nc.tensor.matmul(
                            pd[:],
                            actL[s_][:, ip, :],
                            w2L_t[:, ip, hch * 512 : (hch + 1) * 512],
                            start=(ip == 0),
                            stop=(ip == NI - 1),
                        )
                    if hch % 2 == 0:
                        nc.vector.tensor_scalar_mul(
                            y[:, hch * 512 : (hch + 1) * 512], pd[:], 1.0 / WS
                        )
                    else:
                        nc.scalar.activation(
                            y[:, hch * 512 : (hch + 1) * 512], pd[:], Copy, 0.0,
                            1.0 / WS,
                        )
                    store(
                        yall[v * P : (v + 1) * P, hch * 512 : (hch + 1) * 512],
                        y[:, hch * 512 : (hch + 1) * 512],
                    )

    nc.compile()
    return nc


def _prepare(inputs):
    """Host gate + dispatch: returns (nc, in_maps) ready for SPMD dispatch."""
    import ml_dtypes

    hidden_states = np.asarray(inputs["hidden_states"], dtype=np.float32)
    gate_w = np.asarray(inputs["gate_w"], dtype=np.float32)
    gate_bias = np.asarray(inputs["gate_bias"], dtype=np.float32)
    w1 = np.asarray(inputs["w1"], dtype=np.float32)
    w2 = np.asarray(inputs["w2"], dtype=np.float32)
    shared_up = np.asarray(inputs["shared_up"], dtype=np.float32)
    shared_down = np.asarray(inputs["shared_down"], dtype=np.float32)

    x = hidden_states.reshape(T, H)

    # ---- host gate + dispatch ----
    top_idx, topw = _gate_numpy(x, gate_w, gate_bias)
    sqw = np.sqrt(topw)

    tok_lists = [[] for _ in range(E)]
    scale_lists = [[] for _ in range(E)]
    for kk in range(K):
        for t in range(T):
            e = top_idx[t, kk]
            tok_lists[e].append(t)
            scale_lists[e].append(sqw[t, kk])
    counts = np.array([len(l) for l in tok_lists])
    nslot = max(1, int(np.ceil(counts.max() / P)))

    if nslot not in _PROG_CACHE:
        _PROG_CACHE[nslot] = _build_program(nslot)
    nc = _PROG_CACHE[nslot]

    NV = EL * nslot
    CAP = nslot * P

    in_maps = []
    for c in range(N_CORES):
        xs_b = np.zeros((NV, P, KH, P), ml_dtypes.float8_e3m4)
        for j in range(EL):
            e = c * EL + j
            toks = np.array(tok_lists[e], dtype=np.int64)
            scls = np.array(scale_lists[e], dtype=np.float32)
            n = len(toks)
            assert n <= CAP
            if n:
                xsp = np.zeros((CAP, H), np.float32)
                xsp[:n] = x[toks] * (scls[:, None] * XS_S)
                # [v within expert, p, k, c] = xsp[128v+c, 128k+p]
                xs_b[j * nslot : (j + 1) * nslot] = (
                    xsp.reshape(nslot, P, KH, P)
                    .transpose(0, 3, 2, 1)
                    .astype(ml_dtypes.float8_e3m4)
                )
        w1c = w1[c * EL : (c + 1) * EL]  # [EL, I, H]
        w2c = w2[c * EL : (c + 1) * EL]  # [EL, H, I]
        # w1 blocked: [e, p, k, ci, i] = w1[e, 128ci+i, 128k+p]
        w1_b = w1c.reshape(EL, NI, P, KH, P).transpose(0, 4, 3, 1, 2)
        # w2 blocked: [e, p, ip, h] = w2[e, h, 128ip+p]
        w2_b = w2c.reshape(EL, H, NI, P).transpose(0, 3, 2, 1)
        # shared 2x4 split: token half th, intermediate quarter q
        th, q = c // 4, c % 4
        cs = q * SQ
        # xt blocked: [p, k, t'] = x[512*th + t', 128k+p]
        xt_b = (
            x[th * T2 : (th + 1) * T2]
            .reshape(T2, KH, P)
            .transpose(2, 1, 0)
            .astype(np.float16)
        )
        # su blocked: [p, k, m, i] = shared_up[cs+128m+i, 128k+p]
        su_b = shared_up[cs : cs + SQ].reshape(NSQ, P, KH, P).transpose(3, 2, 0, 1)
        # sd blocked: [p, j, h] = shared_down[h, cs+128j+p]
        sd_b = shared_down[:, cs : cs + SQ].reshape(H, NSQ, P).transpose(2, 1, 0)
        in_maps.append(
            {
                "xt": np.ascontiguousarray(xt_b),
                "xs": np.ascontiguousarray(xs_b).view(np.uint8),
                "w1": np.ascontiguousarray(
                    (w1_b * WS).astype(ml_dtypes.float8_e3m4)
                ).view(np.uint8),
                "w2": np.ascontiguousarray(
                    (w2_b * WS).astype(ml_dtypes.float8_e3m4)
                ).view(np.uint8),
                "su": np.ascontiguousarray(su_b.astype(np.float16)),
                "sd": np.ascontiguousarray(sd_b.astype(np.float16)),
            }
        )

    return nc, in_maps, tok_lists, nslot


def _combine(results, tok_lists, nslot, out_shape, out_dtype):
    """Host unshard: sum shared partials + scatter-add routed expert rows."""
    CAP = nslot * P
    acc = np.zeros((T, H), np.float32)
    for c in range(N_CORES):
        th = c // 4
        acc[th * T2 : (th + 1) * T2] += results[c]["out"].astype(np.float32)
    for c in range(N_CORES):
        ya = results[c]["yall"].astype(np.float32)
        for j in range(EL):
            toks = tok_lists[c * EL + j]
            n = len(toks)
            if n:
                acc[toks] += ya[j * CAP : j * CAP + n]
    return acc.reshape(out_shape).astype(out_dtype)


def kernel(**inputs):
    from concourse.bass_utils import run_bass_kernel_spmd

    hidden_states = np.asarray(inputs["hidden_states"], dtype=np.float32)
    nc, in_maps, tok_lists, nslot = _prepare(inputs)
    res = run_bass_kernel_spmd(nc, in_maps, list(range(N_CORES)))
    return _combine(
        res.results, tok_lists, nslot, hidden_states.shape, hidden_states.dtype
    )
